# revision 14
# baseline (speedup 1.0000x reference)
"""Trainium2 Bass kernel for a 3-layer GCN (Citeseer-style message passing).

Math (equivalent to reference.py):
    z1 = x @ w1
    h1 = relu(A @ z1 + b1)          # A: sparse 50000x50000, 800K weighted edges
    z2 = h1 @ w2
    h2 = relu(A @ z2 + b2)
    out = log_softmax((A @ h2) @ w3 + b3)   # == A @ (h2 @ w3) by linearity

Distribution: nodes row-sharded across 8 NeuronCores. Each dense/sparse stage
computes the local row shard; AllGather replicates the activations that the
next sparse aggregation must gather from (halo exchange degenerates to full
replication because the edge list is random).

Sparse aggregation on TensorE: edges are sorted by destination, grouped per
128-destination-node tile into 128-edge blocks. For each block the host
precomputes a selection matrix S[e, n] = w_e * (dst_local[e] == n). Then
    agg[tile] = sum_blocks S_blk^T @ z[src_blk]       (PSUM accumulation)
with z[src_blk] ([128, 512] rows) fetched by gpsimd.dma_gather (the Ant Q7
row-gather). dma_gather takes int16 indices, so the replicated activation
table is addressed in two halves split at row 32768; every (tile, half) edge
list is padded to a uniform block count so the SPMD program is data-independent.
"""

import os
import sys

for _p in ("/opt/trn_rl_repo",):
    if _p not in sys.path:
        sys.path.insert(0, _p)

import numpy as np

import concourse.bass as bass
import concourse.mybir as mybir
import concourse.tile as tile
from concourse import bacc
from concourse.bass_utils import run_bass_kernel_spmd
from concourse.masks import make_identity

P = 128
HALF = 32768  # int16 index limit for dma_gather


class Cfg:
    def __init__(
        self,
        n_nodes=50000,
        in_dim=3703,
        hidden=500,
        classes=6,
        n_cores=8,
        nbt=18,          # edge blocks per destination tile (uniform, padded)
        gchunk=6,        # blocks per indirect-DMA gather
        dt="float32",    # dtype for z tables / S matrices / gathered msgs
    ):
        self.n_nodes = n_nodes
        self.in_dim = in_dim
        self.hidden = hidden
        self.classes = classes
        self.n_cores = n_cores
        assert n_nodes % n_cores == 0
        self.npc_real = n_nodes // n_cores            # real nodes per core
        self.tiles = (self.npc_real + P - 1) // P     # dst tiles per core
        self.npc = self.tiles * P                     # padded nodes per core
        self.kt = (in_dim + P - 1) // P               # k-tiles for layer 1
        self.in_pad = self.kt * P
        self.d = ((hidden + P - 1) // P) * P          # padded hidden
        self.hc = self.d // P                         # hidden 128-chunks
        self.c = 8 if classes <= 8 else ((classes + P - 1) // P) * P
        self.nbt = nbt
        self.gchunk = gchunk
        self.dt = dt

    @property
    def mdt(self):
        return mybir.dt.float32 if self.dt == "float32" else mybir.dt.bfloat16

    @property
    def npdt(self):
        if self.dt == "float32":
            return np.float32
        import ml_dtypes

        return ml_dtypes.bfloat16

    def key(self):
        return (
            self.n_nodes, self.in_dim, self.hidden, self.classes,
            self.n_cores, self.nbt, self.gchunk, self.dt,
        )


# ----------------------------------------------------------------------------
# Host-side preprocessing
# ----------------------------------------------------------------------------

def prepare_inputs(cfg, x, w1, b1, w2, b2, w3, b3, edge_w, edge_src, edge_dst):
    """Shard + pack all inputs. Returns (in_maps, nbt_used)."""
    nc_, npcr, npc, tiles = cfg.n_cores, cfg.npc_real, cfg.npc, cfg.tiles
    f32 = np.float32

    x = np.asarray(x, f32)
    edge_src = np.asarray(edge_src, np.int64)
    edge_dst = np.asarray(edge_dst, np.int64)
    edge_w = np.asarray(edge_w, f32)

    # --- dense weights (replicated) ---
    w1p = np.zeros((cfg.in_pad, cfg.d), f32)
    w1p[: cfg.in_dim, : cfg.hidden] = np.asarray(w1, f32)
    w1p = np.ascontiguousarray(w1p.reshape(cfg.kt, P, cfg.d))

    w2p = np.zeros((cfg.d, cfg.d), f32)
    w2p[: cfg.hidden, : cfg.hidden] = np.asarray(w2, f32)
    w2p = np.ascontiguousarray(w2p.reshape(cfg.hc, P, cfg.d)).astype(cfg.npdt)

    w3p = np.zeros((cfg.d, cfg.c), f32)
    w3p[: cfg.hidden, : cfg.classes] = np.asarray(w3, f32)
    w3p = np.ascontiguousarray(w3p.reshape(cfg.hc, P, cfg.c)).astype(cfg.npdt)

    def rep_bias(b, width, pad_val=0.0):
        bp = np.full((width,), pad_val, f32)
        bp[: b.shape[0]] = np.asarray(b, f32)
        return np.ascontiguousarray(np.broadcast_to(bp, (P, width)).copy())

    b1p = rep_bias(b1, cfg.d)
    b2p = rep_bias(b2, cfg.d)
    b3p = rep_bias(b3, cfg.c, pad_val=-1e30)

    # --- edge prep ---
    # zfull row index for a global node id
    src_row = (edge_src // npcr) * npc + (edge_src % npcr)
    dst_core = edge_dst // npcr
    dst_loc = edge_dst % npcr
    dst_tile = dst_loc // P
    dst_in_tile = dst_loc % P

    # group edges by (core, tile, table-half); uniform block counts per group
    is_high = (src_row >= HALF).astype(np.int64)
    flat_tile = dst_core * tiles + dst_tile
    grp = flat_tile * 2 + is_high
    ngrp = nc_ * tiles * 2
    counts = np.bincount(grp, minlength=ngrp)
    nbl = max(1, int(np.max((counts[0::2] + P - 1) // P)))
    nbh = int(np.max((counts[1::2] + P - 1) // P))
    nb2 = nbl + nbh
    cap2 = nb2 * P

    # sort by (group, src_row) for HBM locality within a group
    order = np.lexsort((src_row, grp))
    grp_s = grp[order]
    src_s = src_row[order].astype(np.int32)
    src_s = np.where(is_high[order] > 0, src_s - HALF, src_s)
    dil_s = dst_in_tile[order].astype(np.int32)
    w_s = edge_w[order]

    # slot of each edge within its (core,tile) capacity region
    starts = np.zeros(ngrp + 1, np.int64)
    np.cumsum(counts, out=starts[1:])
    within = np.arange(len(order)) - starts[grp_s]
    region = (grp_s // 2) * cap2 + (grp_s % 2) * (nbl * P)
    slot = region + within

    gidx_all = np.zeros((nc_, tiles, cap2), np.int32)
    sm_all = np.zeros((nc_, tiles, cap2, P), f32)
    gidx_flat = gidx_all.reshape(-1)
    sm_flat = sm_all.reshape(-1, P)
    gidx_flat[slot] = src_s
    sm_flat[slot, dil_s] = w_s

    assert gidx_all.max() < HALF
    # wrap indices for dma_gather: idx j of a block list lives at
    # [partition j%16, col j//16], replicated across the 8 Q7 cores
    g16 = gidx_all.reshape(nc_, tiles, nb2 * 8, 16).astype(np.int16)
    gidx16 = np.tile(
        g16.transpose(0, 3, 1, 2), (1, 8, 1, 1)
    )  # [nc, 128, tiles, nb2*8]
    sm_all = sm_all.reshape(nc_, tiles, nb2, P, P).astype(cfg.npdt)

    identity = np.eye(P, dtype=np.float32).astype(cfg.npdt)

    # --- X^T shards ---
    in_maps = []
    for c in range(nc_):
        xc = np.zeros((npc, cfg.in_pad), f32)
        xc[:npcr, : cfg.in_dim] = x[c * npcr : (c + 1) * npcr]
        xt = np.ascontiguousarray(xc.T).reshape(cfg.kt, P, npc)
        in_maps.append(
            {
                "xt": xt,
                "w1": w1p,
                "w2": w2p,
                "w3": w3p,
                "b1": b1p,
                "b2": b2p,
                "b3": b3p,
                "ident": identity,
                "smat": np.ascontiguousarray(sm_all[c]),
                "gidx": np.ascontiguousarray(gidx16[c]),
            }
        )
    return in_maps, (nbl, nbh)


# ----------------------------------------------------------------------------
# Bass program
# ----------------------------------------------------------------------------

def build_program(cfg, nbt, debug=False):
    nbl, nbh = nbt
    nb2 = nbl + nbh
    f32 = mybir.dt.float32
    DT = cfg.mdt
    tiles, kt, d, hc, c = cfg.tiles, cfg.kt, cfg.d, cfg.hc, cfg.c
    npc = cfg.npc
    nfull = cfg.n_cores * npc
    groups = [list(range(cfg.n_cores))]

    nc = bacc.Bacc(
        "TRN2",
        target_bir_lowering=False,
        debug=debug,
        num_devices=cfg.n_cores,
    )

    xt = nc.dram_tensor("xt", [kt, P, npc], f32, kind="ExternalInput")
    w1 = nc.dram_tensor("w1", [kt, P, d], f32, kind="ExternalInput")
    w2 = nc.dram_tensor("w2", [hc, P, d], DT, kind="ExternalInput")
    w3 = nc.dram_tensor("w3", [hc, P, c], DT, kind="ExternalInput")
    b1 = nc.dram_tensor("b1", [P, d], f32, kind="ExternalInput")
    b2 = nc.dram_tensor("b2", [P, d], f32, kind="ExternalInput")
    b3 = nc.dram_tensor("b3", [P, c], f32, kind="ExternalInput")
    ident_in = nc.dram_tensor("ident", [P, P], DT, kind="ExternalInput")
    smat = nc.dram_tensor("smat", [tiles, nb2, P, P], DT, kind="ExternalInput")
    gidx = nc.dram_tensor(
        "gidx", [P, tiles, nb2 * 8], mybir.dt.int16, kind="ExternalInput"
    )
    # output laid out [P, tiles, c]; host un-permutes (row = t*128 + p)
    out = nc.dram_tensor("out", [P, tiles, c], f32, kind="ExternalOutput")

    # gather chunking: (block_start, block_end, is_high) with uniform layout
    chunks = []
    for base, nb in ((0, nbl), (nbl, nbh)):
        b0 = 0
        while b0 < nb:
            b1_ = min(b0 + cfg.gchunk, nb)
            chunks.append((base + b0, base + b1_, base > 0))
            b0 = b1_

    with tile.TileContext(nc) as tc:
        with tc.tile_pool(name="dram", bufs=1, space="DRAM") as dp:
            z1s = dp.tile([npc, d], DT)
            z1f = dp.tile([nfull, d], DT, addr_space="Shared")
            z2s = dp.tile([npc, d], DT)
            z2f = dp.tile([nfull, d], DT, addr_space="Shared")
            h2s = dp.tile([npc, d], DT)
            h2f = dp.tile([nfull, d], DT, addr_space="Shared")

            with tc.tile_pool(name="consts", bufs=1) as cp:
                w2_sb = cp.tile([P, hc, d], DT)
                nc.sync.dma_start(out=w2_sb[:], in_=w2[:].rearrange("k p d -> p k d"))
                w3_sb = cp.tile([P, hc, c], DT)
                nc.sync.dma_start(out=w3_sb[:], in_=w3[:].rearrange("k p d -> p k d"))
                b1_sb = cp.tile([P, d], f32)
                nc.sync.dma_start(out=b1_sb[:], in_=b1[:])
                b2_sb = cp.tile([P, d], f32)
                nc.sync.dma_start(out=b2_sb[:], in_=b2[:])
                b3_sb = cp.tile([P, c], f32)
                nc.sync.dma_start(out=b3_sb[:], in_=b3[:])
                ident = cp.tile([P, P], DT)
                nc.sync.dma_start(out=ident[:], in_=ident_in[:])
                idx_sb = cp.tile([P, tiles, nb2 * 8], mybir.dt.int16)
                nc.sync.dma_start(out=idx_sb[:], in_=gidx[:])

                # ------------- Phase A: z1 = x @ w1 -------------
                with (
                    tc.tile_pool(name="pa_w", bufs=1) as wp,
                    tc.tile_pool(name="pa_x", bufs=3) as xp,
                    tc.tile_pool(name="pa_ps", bufs=2, space="PSUM") as pp,
                    tc.tile_pool(name="pa_o", bufs=3) as op_,
                ):
                    w1_sb = wp.tile([P, kt, d], f32)
                    nc.sync.dma_start(
                        out=w1_sb[:], in_=w1[:].rearrange("k p d -> p k d")
                    )
                    for t in range(tiles):
                        xt_t = xp.tile([P, kt, P], f32, tag="xt")
                        nc.sync.dma_start(
                            out=xt_t[:],
                            in_=xt[:, :, t * P : (t + 1) * P].rearrange(
                                "k p m -> p k m"
                            ),
                        )
                        ps = pp.tile([P, d], f32, space="PSUM", tag="ps")
                        for k in range(kt):
                            nc.tensor.matmul(
                                ps[:],
                                lhsT=xt_t[:, k, :],
                                rhs=w1_sb[:, k, :],
                                start=(k == 0),
                                stop=(k == kt - 1),
                            )
                        zo = op_.tile([P, d], DT, tag="zo")
                        nc.vector.tensor_copy(out=zo[:], in_=ps[:])
                        nc.sync.dma_start(
                            out=z1s[t * P : (t + 1) * P, :], in_=zo[:]
                        )

                nc.gpsimd.collective_compute(
                    "AllGather",
                    mybir.AluOpType.bypass,
                    replica_groups=groups,
                    ins=[z1s[:].opt()],
                    outs=[z1f[:].opt()],
                )

                # ------------- generic aggregation phase -------------
                def agg_phase(zfull, bias_sb, relu, out_fn, name):
                    with (
                        tc.tile_pool(name=f"{name}_s", bufs=2) as sp,
                        tc.tile_pool(name=f"{name}_m", bufs=3) as mp,
                        tc.tile_pool(name=f"{name}_ps", bufs=2, space="PSUM") as pp,
                        tc.tile_pool(name=f"{name}_h", bufs=2) as hp,
                        tc.tile_pool(name=f"{name}_aux", bufs=2) as ap_,
                        tc.tile_pool(name=f"{name}_tps", bufs=2, space="PSUM") as tpp,
                        tc.tile_pool(name=f"{name}_o", bufs=3) as op_,
                    ):
                        for t in range(tiles):
                            s_sb = sp.tile([P, nb2, P], DT, tag="s")
                            nc.sync.dma_start(
                                out=s_sb[:],
                                in_=smat[t].rearrange("b e n -> e b n"),
                            )
                            agg_ps = pp.tile([P, d], f32, space="PSUM", tag="agg")
                            for g0, g1, hi in chunks:
                                gc = g1 - g0
                                msgs = mp.tile(
                                    [P, cfg.gchunk, d], DT, tag="msgs"
                                )
                                nc.gpsimd.dma_gather(
                                    msgs[:, :gc, :],
                                    zfull[HALF:, :] if hi else zfull[:, :],
                                    idx_sb[:, t, g0 * 8 : g1 * 8],
                                    gc * P,
                                    gc * P,
                                    d,
                                )
                                for j in range(gc):
                                    b = g0 + j
                                    nc.tensor.matmul(
                                        agg_ps[:],
                                        lhsT=s_sb[:, b, :],
                                        rhs=msgs[:, j, :],
                                        start=(b == 0),
                                        stop=(b == nb2 - 1),
                                    )
                            h_sb = hp.tile([P, d], DT, tag="h")
                            if relu:
                                tmp = ap_.tile([P, d], f32, tag="tmp")
                                nc.vector.tensor_tensor(
                                    out=tmp[:],
                                    in0=agg_ps[:],
                                    in1=bias_sb[:],
                                    op=mybir.AluOpType.add,
                                )
                                nc.scalar.activation(
                                    out=h_sb[:],
                                    in_=tmp[:],
                                    func=mybir.ActivationFunctionType.Relu,
                                )
                            else:
                                nc.vector.tensor_copy(out=h_sb[:], in_=agg_ps[:])
                            out_fn(t, h_sb, tpp, pp, op_, ap_)

                # transpose h tile -> hT [P, hc*P]
                def transpose_h(h_sb, tpp, ap_):
                    hT = ap_.tile([P, hc, P], DT, tag="ht")
                    for cc in range(hc):
                        tps = tpp.tile([P, P], f32, space="PSUM", tag="tps")
                        nc.tensor.transpose(
                            out=tps[:],
                            in_=h_sb[:, cc * P : (cc + 1) * P],
                            identity=ident[:],
                        )
                        nc.vector.tensor_copy(out=hT[:, cc, :], in_=tps[:])
                    return hT

                # Phase B: h1 = relu(agg(z1)+b1); z2 = h1 @ w2
                def b_out(t, h_sb, tpp, pp, op_, ap_):
                    hT = transpose_h(h_sb, tpp, ap_)
                    zps = pp.tile([P, d], f32, space="PSUM", tag="zps")
                    for cc in range(hc):
                        nc.tensor.matmul(
                            zps[:],
                            lhsT=hT[:, cc, :],
                            rhs=w2_sb[:, cc, :],
                            start=(cc == 0),
                            stop=(cc == hc - 1),
                        )
                    zo = op_.tile([P, d], DT, tag="zo")
                    nc.vector.tensor_copy(out=zo[:], in_=zps[:])
                    nc.sync.dma_start(out=z2s[t * P : (t + 1) * P, :], in_=zo[:])

                agg_phase(z1f, b1_sb, True, b_out, "pb")

                nc.gpsimd.collective_compute(
                    "AllGather",
                    mybir.AluOpType.bypass,
                    replica_groups=groups,
                    ins=[z2s[:].opt()],
                    outs=[z2f[:].opt()],
                )

                # Phase C: h2 = relu(agg(z2)+b2)
                def c_out(t, h_sb, tpp, pp, op_, ap_):
                    nc.sync.dma_start(out=h2s[t * P : (t + 1) * P, :], in_=h_sb[:])

                agg_phase(z2f, b2_sb, True, c_out, "pc")

                nc.gpsimd.collective_compute(
                    "AllGather",
                    mybir.AluOpType.bypass,
                    replica_groups=groups,
                    ins=[h2s[:].opt()],
                    outs=[h2f[:].opt()],
                )

                # Phase D: out = log_softmax(agg(h2) @ w3 + b3)
                with tc.tile_pool(name="pd_zout", bufs=1) as zp:
                    zout = zp.tile([P, tiles, c], f32)

                    def d_out(t, h_sb, tpp, pp, op_, ap_):
                        hT = transpose_h(h_sb, tpp, ap_)
                        zps = pp.tile([P, c], f32, space="PSUM", tag="z3ps")
                        for cc in range(hc):
                            nc.tensor.matmul(
                                zps[:],
                                lhsT=hT[:, cc, :],
                                rhs=w3_sb[:, cc, :],
                                start=(cc == 0),
                                stop=(cc == hc - 1),
                            )
                        nc.vector.tensor_tensor(
                            out=zout[:, t, :],
                            in0=zps[:],
                            in1=b3_sb[:],
                            op=mybir.AluOpType.add,
                        )

                    agg_phase(h2f, None, False, d_out, "pd")

                    # log-softmax over class dim (innermost of zout)
                    with tc.tile_pool(name="sm", bufs=1) as smp:
                        mx = smp.tile([P, tiles], f32)
                        nc.vector.tensor_reduce(
                            out=mx[:],
                            in_=zout[:],
                            axis=mybir.AxisListType.X,
                            op=mybir.AluOpType.max,
                        )
                        sh = smp.tile([P, tiles, c], f32)
                        nc.vector.tensor_tensor(
                            out=sh[:],
                            in0=zout[:],
                            in1=mx[:, :, None].to_broadcast([P, tiles, c]),
                            op=mybir.AluOpType.subtract,
                        )
                        ex = smp.tile([P, tiles, c], f32)
                        nc.scalar.activation(
                            out=ex[:],
                            in_=sh[:],
                            func=mybir.ActivationFunctionType.Exp,
                        )
                        sm_ = smp.tile([P, tiles], f32)
                        nc.vector.tensor_reduce(
                            out=sm_[:],
                            in_=ex[:],
                            axis=mybir.AxisListType.X,
                            op=mybir.AluOpType.add,
                        )
                        lg = smp.tile([P, tiles], f32)
                        nc.scalar.activation(
                            out=lg[:],
                            in_=sm_[:],
                            func=mybir.ActivationFunctionType.Ln,
                        )
                        res = smp.tile([P, tiles, c], f32)
                        nc.vector.tensor_tensor(
                            out=res[:],
                            in0=sh[:],
                            in1=lg[:, :, None].to_broadcast([P, tiles, c]),
                            op=mybir.AluOpType.subtract,
                        )
                        nc.sync.dma_start(out=out[:], in_=res[:])

    nc.compile()
    return nc


# ----------------------------------------------------------------------------
# Entry point
# ----------------------------------------------------------------------------

_PROGRAM_CACHE = {}


def _get_program(cfg, nbt):
    key = (cfg.key(), nbt)
    if key not in _PROGRAM_CACHE:
        _PROGRAM_CACHE[key] = build_program(cfg, nbt)
    return _PROGRAM_CACHE[key]


def run(cfg, inputs, trace=False, trace_kwargs=None):
    """Full pipeline: prep -> build -> run on 8 cores -> unshard.

    Returns (output, bass_results).
    """
    in_maps, nbt = prepare_inputs(cfg, **inputs)
    nc = _get_program(cfg, nbt)
    res = run_bass_kernel_spmd(
        nc,
        in_maps,
        core_ids=list(range(cfg.n_cores)),
        trace=trace,
        **(trace_kwargs or {}),
    )
    outs = [res.results[i]["out"] for i in range(cfg.n_cores)]
    # device layout [P, tiles, c] -> [npc, c]; drop padding
    full = np.concatenate(
        [
            np.transpose(o, (1, 0, 2)).reshape(cfg.npc, -1)[
                : cfg.npc_real, : cfg.classes
            ]
            for o in outs
        ],
        axis=0,
    )
    return np.ascontiguousarray(full.astype(np.float32)), res


def kernel(**inputs):
    cfg = Cfg()
    out, _ = run(cfg, inputs)
    return out
